# revision 14
# baseline (speedup 1.0000x reference)
"""Trainium2 Bass kernel for nn_Encoder_conv_mlp (GNN message passing encoder).

Reference computation (per graph batch):
    h1 = relu(segsum(x[src]->dst) @ W1_rel.T + x @ W1_root.T + b1)
    h2 = relu(segsum(h1[src]->dst) @ W2_rel.T + h1 @ W2_root.T + b2)
    hb = h2.reshape(bs, 64*256)
    mu = hb @ Wmu.T + bmu ; logvar = hb @ Wlv.T + blv

Sharding: data-parallel over graphs. 512 graphs / 8 cores = 64 graphs
(4096 nodes, 65536 edges) per core. Edges never cross graphs, so each
core is fully independent; weights are replicated and the host simply
concatenates the per-core outputs.

Message passing is dense matmuls against per-2-graph-block adjacency
count matrices A2T[s, d] = #edges(src=s -> dst=d), shipped as fp8e4
(counts are small ints, exact in fp8 -> half the DMA bytes).

Layer 1 aggregates BEFORE projecting (the input space is 128-dim vs the
256-dim hidden space): agg_x = A.T x costs half the PE cycles of
aggregating projected features, and its eviction traffic is half of the
projected-first hr eviction. This needs x in node-major layout for the
aggregation and feature-major for the root projection, so x ships in
both layouts (DMA has slack; PE is the critical path). Layer-1
evictions are spread over three engines (Pool: agg psum copy, ACT: relu
mo=0, DVE: relu mo=1) so no single engine limits the shortened L1.

Layer 2 projects first (h1's rel-projection lands node-major for free
as the aggregation lhsT), as in the reference PyG ordering.

The readout is computed latent-major: pro[l, g] += wro_tile.T @ h2_slice
with free dim 64 (graphs) instead of 256 -> half the PE cycles of the
graph-major orientation. The [bmu|blv] bias folds in as a K=1 matmul
against a ones vector, and the psum is evicted with a single ACT copy
feeding one output DMA.

Scheduling: f32 biases/w1 ride packed inside the bf16 input tensors
(bitcast views); per-group combined DMA chunks [xT_g | xnm_g | a2t_g]
keep the DMA count low (HWDGE issue cadence is ~625ns/transfer); a
warm-up matmul stream covers the PE clock ramp (HAM) while the first
input DMAs land; L2 runs all rel-projections first, then the whole mo=0
pass before mo=1 so h2's first feature half completes early; the
readout orders k-tiles so early-arriving wro chunks are consumed first
and only even k-tiles (gated by h2_sb[0]) run at its head.
"""
import sys

if "/opt/trn_rl_repo" not in sys.path:
    sys.path.insert(0, "/opt/trn_rl_repo")

import numpy as np
import ml_dtypes

N_NODES = 64
BS = 512
IN_F = 128
HID = 256
LAT = 128
N_CORES = 8
G_PER = BS // N_CORES          # 64 graphs per core
NODES_PER = G_PER * N_NODES    # 4096 nodes per core
BLOCKS = NODES_PER // 128      # 32 two-graph blocks per core
GROUPS = NODES_PER // 512      # 8 512-node groups per core
KT = (N_NODES * HID) // 128    # 128 readout contraction tiles

BF16 = ml_dtypes.bfloat16
F8E4 = ml_dtypes.float8_e4m3

_PROGRAM = None


def _build_program():
    import concourse.bacc as bacc
    import concourse.mybir as mybir
    import concourse.tile as tile

    nc = bacc.Bacc("TRN2", target_bir_lowering=False, debug=False,
                   num_devices=N_CORES)
    BF = mybir.dt.bfloat16
    F32 = mybir.dt.float32
    F8 = mybir.dt.float8e4

    # xw = [w1_pack 520 | xT_g0 512] [xnm_g0 512 | a2t_g0 256]
    #      then per g in 1..7: [xT_g 512 | xnm_g 512 | a2t_g 256]
    xw = nc.dram_tensor("xw", [128, 1800 + 7 * 1280], BF,
                        kind="ExternalInput").ap()
    w2 = nc.dram_tensor("w2", [128, 1280], BF, kind="ExternalInput").ap()
    wro = nc.dram_tensor("wro", [128, KT * 256], BF, kind="ExternalInput").ap()
    out = nc.dram_tensor("out", [128, 128], F32, kind="ExternalOutput").ap()

    Relu = mybir.ActivationFunctionType.Relu
    Copy = mybir.ActivationFunctionType.Copy
    Add = mybir.AluOpType.add
    Max = mybir.AluOpType.max

    with tile.TileContext(nc) as tc:
        with (
            tc.tile_pool(name="const", bufs=1) as const,
            tc.tile_pool(name="hr", bufs=20) as hr_pool,
            tc.tile_pool(name="psum_hr", bufs=3, space="PSUM") as psum_hr,
            tc.tile_pool(name="psum_fm", bufs=4, space="PSUM") as psum_fm,
            tc.tile_pool(name="psum_ro", bufs=1, space="PSUM") as psum_ro,
        ):
            # Per-chunk tiles so each consumer depends only on its chunk's DMA.
            lead_sb = const.tile([128, 1032], BF, tag="lead_sb")
            g0b_sb = const.tile([128, 768], BF, tag="g0b_sb")
            gch_sb = [const.tile([128, 1280], BF, name=f"gch{g}", tag=f"gch{g}")
                      for g in range(1, 8)]
            w2_sb = const.tile([128, 1280], BF, tag="w2_sb")
            wro_sb = [const.tile([128, 4096], BF, name=f"wro{i}", tag=f"wro{i}")
                      for i in range(8)]
            # h1 split per (ko, group) for L1->L2 pipelining; h2 per ko chunk.
            h1_sb = [[const.tile([128, 512], BF, name=f"h1_{ko}_{g}",
                                 tag=f"h1_{ko}_{g}")
                      for g in range(GROUPS)] for ko in range(2)]
            h2_sb = [const.tile([128, NODES_PER], BF, name=f"h2_{fo}",
                                tag=f"h2_{fo}")
                     for fo in range(2)]

            # DMA issue order = priority order for the head of the kernel.
            nc.sync.dma_start(lead_sb[:], xw[:, 0:1032])
            nc.sync.dma_start(g0b_sb[:], xw[:, 1032:1800])
            for g in range(1, 8):
                nc.sync.dma_start(
                    gch_sb[g - 1][:],
                    xw[:, 1800 + (g - 1) * 1280:1800 + g * 1280])
            nc.sync.dma_start(w2_sb[:], w2[:])
            for i in range(8):
                nc.sync.dma_start(wro_sb[i][:], wro[:, i * 4096:(i + 1) * 4096])

            # w1 + biases ride packed inside lead/w2 (bitcast views for f32)
            w1_sb = lead_sb[:, 0:520]
            b12_sb = lead_sb[:, 512:520].bitcast(F32)
            # readout bias [bmu | blv] as a bf16 row vector (K=1 matmul lhsT)
            bias_ro_sb = w2_sb[0:1, 1024:1280]

            # PE pre-warm: dummy matmuls on memset data keep the PE busy from
            # ~1.1us so the clock ramp (HAM) completes before the first real
            # matmul arrives behind the input DMAs (~3.6us); the count is
            # tuned so the warm stream ends just as the real one begins.
            N_WARM = 22
            ones_sb = const.tile([1, 320], BF, tag="ones_sb")
            nc.gpsimd.memset(ones_sb[:], 1.0)
            warm = psum_ro.tile([128, 128], F32, tag="pro")
            for i in range(N_WARM):
                nc.tensor.matmul(warm[0:64, :], lhsT=ones_sb[:, 256:320],
                                 rhs=ones_sb[:, 0:128],
                                 start=(i == 0), stop=(i == N_WARM - 1))

            def x_cols(c0, c1):        # feature-major x slice [128, c1-c0]
                g = c0 // 512
                assert c1 <= (g + 1) * 512
                if g == 0:
                    return lead_sb[:, 520 + c0:520 + c1]
                return gch_sb[g - 1][:, c0 - g * 512:c1 - g * 512]

            def xnm_cols(b):           # node-major x block [128 n, 128 f]
                g = b // 4
                off = (b % 4) * 128
                if g == 0:
                    return g0b_sb[:, off:off + 128]
                return gch_sb[g - 1][:, 512 + off:512 + off + 128]

            def a2t_blk(b):            # [128, 128] fp8 adjacency for block b
                g = b // 4
                off = (b % 4) * 128
                if g == 0:
                    base = g0b_sb[:, 512:768].bitcast(F8)
                else:
                    base = gch_sb[g - 1][:, 1024:1280].bitcast(F8)
                return base[:, off:off + 128]

            # ---- Layer 1: aggregate-first ----
            # agg_x[f, d] = sum_s x[s, f] A[s, d] (128-dim input space),
            # then h1 = relu(W1_rel agg_x + W1_root x + b1) feature-major.
            for grp in range(GROUPS):
                pf = [psum_fm.tile([128, 512], F32, name=f"pf1_{grp}_{mo}",
                                   tag="pf")
                      for mo in range(2)]
                for mo in range(2):
                    nc.tensor.matmul(
                        pf[mo][:],
                        lhsT=w1_sb[:, 256 + mo * 128:256 + (mo + 1) * 128],
                        rhs=x_cols(grp * 512, (grp + 1) * 512),
                        start=True, stop=False, skip_group_check=True,
                    )
                pagg = psum_hr.tile([128, 512], F32, tag="ph")
                for blk in range(4):
                    b = grp * 4 + blk
                    nc.tensor.matmul(
                        pagg[:, blk * 128:(blk + 1) * 128],
                        lhsT=xnm_cols(b), rhs=a2t_blk(b),
                        start=True, stop=True, skip_group_check=True,
                    )
                agg_sb = hr_pool.tile([128, 512], BF, tag="hr")
                nc.gpsimd.tensor_copy(agg_sb[:], pagg[:])
                for mo in range(2):
                    nc.tensor.matmul(
                        pf[mo][:],
                        lhsT=w1_sb[:, mo * 128:(mo + 1) * 128],
                        rhs=agg_sb[:],
                        start=False, stop=True, skip_group_check=True,
                    )
                # relu+bias evictions split across ACT (mo=0) and DVE (mo=1)
                nc.scalar.activation(h1_sb[0][grp][:], pf[0][:], Relu,
                                     bias=b12_sb[:, 0:1])
                nc.vector.tensor_scalar(h1_sb[1][grp][:], pf[1][:],
                                        b12_sb[:, 1:2], 0.0, Add, Max)

            # ---- Layer 2: project-first ----
            def act_cols(ko, c0, c1):
                return h1_sb[ko][c0 // 512][:, c0 % 512:c0 % 512 + (c1 - c0)]

            def emit_hr(grp):
                # two blocks share one [128,512] psum tile so one DVE copy
                # evicts both -> half the copy count
                hrs = []
                for pair in range(2):
                    ph = psum_hr.tile([128, 512], F32, tag="ph")
                    for sub in range(2):
                        b = grp * 4 + pair * 2 + sub
                        for ko in range(2):
                            nc.tensor.matmul(
                                ph[:, sub * 256:(sub + 1) * 256],
                                lhsT=act_cols(ko, b * 128, (b + 1) * 128),
                                rhs=w2_sb[:, ko * 512:ko * 512 + 256],
                                start=(ko == 0), stop=(ko == 1),
                                skip_group_check=True,
                            )
                    hr = hr_pool.tile([128, 512], BF, tag="hr")
                    nc.vector.tensor_copy(hr[:], ph[:])
                    hrs.append(hr)
                return hrs

            def emit_fm(grp, mo, hrs):
                pf = psum_fm.tile([128, 512], F32, name="pf", tag="pf")
                for ko in range(2):
                    nc.tensor.matmul(
                        pf[:],
                        lhsT=w2_sb[:, ko * 512 + 256 + mo * 128:
                                   ko * 512 + 256 + (mo + 1) * 128],
                        rhs=act_cols(ko, grp * 512, (grp + 1) * 512),
                        start=(ko == 0), stop=False,
                        skip_group_check=True,
                    )
                for blk in range(4):
                    b = grp * 4 + blk
                    nc.tensor.matmul(
                        pf[:, blk * 128:(blk + 1) * 128],
                        lhsT=hrs[blk // 2][:, (blk % 2) * 256 + mo * 128:
                                           (blk % 2) * 256 + (mo + 1) * 128],
                        rhs=a2t_blk(b),
                        start=False, stop=(blk == 3),
                        skip_group_check=True,
                    )
                nc.scalar.activation(
                    h2_sb[mo][:, grp * 512:(grp + 1) * 512], pf[:], Relu,
                    bias=b12_sb[:, 2 + mo:3 + mo],
                )

            # L2: all hr projections first, then the whole mo=0 pass before
            # mo=1 — h2_sb[0] (which gates the readout's even k-tiles)
            # completes while the PE still has the entire mo=1 pass queued.
            all_hrs = [emit_hr(grp) for grp in range(GROUPS)]
            for mo in range(2):
                for grp in range(GROUPS):
                    emit_fm(grp, mo, all_hrs[grp])

            # ---- Readout (latent-major) ----
            # pro[l, g] += sum_f wro[ft*128+f, l] * h2_fm[fo][f, g*64+n]
            # Output [128 latent, 64 graphs] per half: free dim 64 instead of
            # 256 -> half the PE cycles of the graph-major orientation.
            # Bias is folded in as a K=1 matmul against a ones vector.
            pro = psum_ro.tile([128, 128], F32, tag="pro")
            for lh in range(2):
                nc.tensor.matmul(
                    pro[:, lh * 64:(lh + 1) * 64],
                    lhsT=bias_ro_sb[:, lh * 128:(lh + 1) * 128],
                    rhs=ones_sb[0:1, 0:64],
                    start=True, stop=False, skip_group_check=True,
                )
            # ft order: (a) even ft (fo=0) of the early chunks first so the
            # readout only waits on h2_sb[0] at its head; (b) chunks 6-7 (the
            # last wro DMA arrivals) last, both halves together.
            fts = [ft for ft in range(96) if ft % 2 == 0] + \
                  [ft for ft in range(96) if ft % 2 == 1] + \
                  list(range(96, KT))
            n_mm = [0, 0]
            for ft in fts:
                n, fo = ft // 2, ft % 2
                rhs = h2_sb[fo][:, n:n + (G_PER - 1) * N_NODES + 1:N_NODES]
                for lh in range(2):
                    n_mm[lh] += 1
                    nc.tensor.matmul(
                        pro[:, lh * 64:(lh + 1) * 64],
                        lhsT=wro_sb[ft // 16][:, ((ft % 16) * 2 + lh) * 128:
                                              ((ft % 16) * 2 + lh + 1) * 128],
                        rhs=rhs,
                        start=False, stop=(n_mm[lh] == KT),
                        skip_group_check=True,
                    )
            out_sb = const.tile([128, 128], F32, tag="out_sb")
            nc.scalar.activation(out_sb[:], pro[:], Copy)
            nc.sync.dma_start(out[:], out_sb[:])

    nc.compile()
    return nc


def _get_program():
    global _PROGRAM
    if _PROGRAM is None:
        _PROGRAM = _build_program()
    return _PROGRAM


def make_in_maps(x, W1_rel, W1_root, b1, W2_rel, W2_root, b2,
                 Wmu, bmu, Wlv, blv, edge_index, batch):
    """Host-side shard + layout prep. Returns per-core input dicts."""
    x = np.asarray(x, dtype=np.float32)
    edge_index = np.asarray(edge_index)

    b12 = np.stack(
        [np.asarray(b1)[0:128], np.asarray(b1)[128:256],
         np.asarray(b2)[0:128], np.asarray(b2)[128:256]], axis=1
    ).astype(np.float32)
    w1_pack = np.concatenate(
        [np.concatenate([np.asarray(W1_rel).T, np.asarray(W1_root).T],
                        axis=1).astype(BF16),
         np.ascontiguousarray(b12).view(BF16)], axis=1)
    w2rT = np.asarray(W2_rel).T.astype(np.float32)
    w2tT = np.asarray(W2_root).T.astype(np.float32)
    bias_pack = np.zeros((128, 256), BF16)
    bias_pack[0, 0:256] = np.concatenate(
        [np.asarray(bmu), np.asarray(blv)]).astype(BF16)
    w2 = np.concatenate(
        [np.concatenate([w2rT[0:128], w2tT[0:128]], axis=1).astype(BF16),
         np.concatenate([w2rT[128:256], w2tT[128:256]], axis=1).astype(BF16),
         bias_pack], axis=1)
    # [128 f, KT*2*128]: per-(ft, lh) stationary [128, 128] tiles, ft-major
    wro_cat = np.concatenate([np.asarray(Wmu).T, np.asarray(Wlv).T], axis=1)
    wro = np.ascontiguousarray(
        wro_cat.reshape(KT, 128, 2, 128).transpose(1, 0, 2, 3)
        .reshape(128, KT * 256)
    ).astype(BF16)

    # Dense per-2-graph-block adjacency counts: A[blk][s, d] = #edges s->d.
    src = edge_index[0].astype(np.int64)
    dst = edge_index[1].astype(np.int64)
    blk = dst >> 7                       # 128 nodes per 2-graph block
    s_loc = src - (blk << 7)
    d_loc = dst - (blk << 7)
    # edges are intra-graph by construction; fail loudly rather than let a
    # cross-block index wrap around in np.add.at
    assert s_loc.min() >= 0 and s_loc.max() < 128, "edge crosses graph block"
    A = np.zeros((BS // 2, 128, 128), np.float32)
    np.add.at(A, (blk, s_loc, d_loc), 1.0)
    assert A.max() <= 16, "edge count not exactly representable in fp8e4"

    in_maps = []
    for c in range(N_CORES):
        xs = x[c * NODES_PER:(c + 1) * NODES_PER]
        xT = np.ascontiguousarray(xs.T).astype(BF16)               # [128, 4096]
        xnm = np.ascontiguousarray(
            xs.reshape(BLOCKS, 128, IN_F).transpose(1, 0, 2)
            .reshape(128, BLOCKS * IN_F)).astype(BF16)             # [128, 4096]
        Ac = A[c * BLOCKS:(c + 1) * BLOCKS]
        a2t8 = np.ascontiguousarray(
            Ac.transpose(1, 0, 2).reshape(128, BLOCKS * 128)
        ).astype(F8E4).view(BF16)                                  # [128, 2048]
        parts = [w1_pack, xT[:, 0:512], xnm[:, 0:512], a2t8[:, 0:256]]
        for g in range(1, 8):
            parts += [xT[:, g * 512:(g + 1) * 512],
                      xnm[:, g * 512:(g + 1) * 512],
                      a2t8[:, g * 256:(g + 1) * 256]]
        xw = np.ascontiguousarray(np.concatenate(parts, axis=1))
        in_maps.append(dict(xw=xw, w2=w2, wro=wro))
    return in_maps


def kernel(**inputs):
    from concourse.bass_utils import run_bass_kernel_spmd

    nc = _get_program()
    in_maps = make_in_maps(**inputs)
    res = run_bass_kernel_spmd(nc, in_maps, list(range(N_CORES)))
    # per-core out is [128 latent, 64 mu-graphs | 64 lv-graphs]
    mu = np.concatenate(
        [res.results[c]["out"][:, 0:G_PER].T for c in range(N_CORES)], axis=0)
    logvar = np.concatenate(
        [res.results[c]["out"][:, G_PER:128].T for c in range(N_CORES)], axis=0)
    return np.ascontiguousarray(mu, np.float32), \
        np.ascontiguousarray(logvar, np.float32)


# revision 16
# speedup vs baseline: 1.0464x; 1.0464x over previous
"""Trainium2 Bass kernel for nn_Encoder_conv_mlp (GNN message passing encoder).

Reference computation (per graph batch):
    h1 = relu(segsum(x[src]->dst) @ W1_rel.T + x @ W1_root.T + b1)
    h2 = relu(segsum(h1[src]->dst) @ W2_rel.T + h1 @ W2_root.T + b2)
    hb = h2.reshape(bs, 64*256)
    mu = hb @ Wmu.T + bmu ; logvar = hb @ Wlv.T + blv

Sharding: data-parallel over graphs. 512 graphs / 8 cores = 64 graphs
(4096 nodes, 65536 edges) per core. Edges never cross graphs, so each
core is fully independent; weights are replicated and the host simply
concatenates the per-core outputs.

Message passing is dense matmuls against per-2-graph-block adjacency
count matrices A2T[s, d] = #edges(src=s -> dst=d), shipped as fp8e4
(counts are small ints, exact in fp8 -> half the DMA bytes).

Layer 1 aggregates BEFORE projecting (the input space is 128-dim vs the
256-dim hidden space): agg_x = A.T x costs half the PE cycles of
aggregating projected features, and its eviction traffic is half of the
projected-first hr eviction. This needs x in node-major layout for the
aggregation and feature-major for the root projection, so x ships in
both layouts (DMA has slack; PE is the critical path). Layer-1
evictions are spread over three engines (Pool: agg psum copy, ACT: relu
mo=0, DVE: relu mo=1) so no single engine limits the shortened L1.

Layer 2 projects first (h1's rel-projection lands node-major for free
as the aggregation lhsT), as in the reference PyG ordering.

The readout is computed latent-major: pro[l, g] += wro_tile.T @ h2_slice
with free dim 64 (graphs) instead of 256 -> half the PE cycles of the
graph-major orientation. The [bmu|blv] bias folds in as a K=1 matmul
against a ones vector, and the psum is evicted with a single ACT copy
feeding one output DMA.

Scheduling: f32 biases/w1 ride packed inside the bf16 input tensors
(bitcast views); per-group combined DMA chunks [xT_g | xnm_g | a2t_g]
keep the DMA count low (HWDGE issue cadence is ~625ns/transfer); a
warm-up matmul stream covers the PE clock ramp (HAM) while the first
input DMAs land; L2 runs all rel-projections first, then the whole mo=0
pass before mo=1 so h2's first feature half completes early; the
readout orders k-tiles so early-arriving wro chunks are consumed first
and only even k-tiles (gated by h2_sb[0]) run at its head.
"""
import sys

if "/opt/trn_rl_repo" not in sys.path:
    sys.path.insert(0, "/opt/trn_rl_repo")

import numpy as np
import ml_dtypes

N_NODES = 64
BS = 512
IN_F = 128
HID = 256
LAT = 128
N_CORES = 8
G_PER = BS // N_CORES          # 64 graphs per core
NODES_PER = G_PER * N_NODES    # 4096 nodes per core
BLOCKS = NODES_PER // 128      # 32 two-graph blocks per core
GROUPS = NODES_PER // 512      # 8 512-node groups per core
KT = (N_NODES * HID) // 128    # 128 readout contraction tiles

BF16 = ml_dtypes.bfloat16
F8E4 = ml_dtypes.float8_e4m3

_PROGRAM = None


def _build_program():
    import concourse.bacc as bacc
    import concourse.mybir as mybir
    import concourse.tile as tile

    nc = bacc.Bacc("TRN2", target_bir_lowering=False, debug=False,
                   num_devices=N_CORES)
    BF = mybir.dt.bfloat16
    F32 = mybir.dt.float32
    F8 = mybir.dt.float8e4

    # xw = [w1_pack 520 | xT_g0 512] [xnm_g0 512 | a2t_g0 256]
    #      then per g in 1..7: [xT_g 512 | xnm_g 512 | a2t_g 256]
    xw = nc.dram_tensor("xw", [128, 1800 + 7 * 1280], BF,
                        kind="ExternalInput").ap()
    w2 = nc.dram_tensor("w2", [128, 1280], BF, kind="ExternalInput").ap()
    wro = nc.dram_tensor("wro", [128, KT * 256], BF, kind="ExternalInput").ap()
    out = nc.dram_tensor("out", [128, 128], F32, kind="ExternalOutput").ap()

    Relu = mybir.ActivationFunctionType.Relu
    Copy = mybir.ActivationFunctionType.Copy
    Add = mybir.AluOpType.add
    Max = mybir.AluOpType.max

    with tile.TileContext(nc) as tc:
        with (
            tc.tile_pool(name="const", bufs=1) as const,
            tc.tile_pool(name="hr", bufs=20) as hr_pool,
            tc.tile_pool(name="psum_hr", bufs=3, space="PSUM") as psum_hr,
            tc.tile_pool(name="psum_fm", bufs=4, space="PSUM") as psum_fm,
            tc.tile_pool(name="psum_ro", bufs=1, space="PSUM") as psum_ro,
        ):
            # Per-chunk tiles so each consumer depends only on its chunk's DMA.
            lead_sb = const.tile([128, 1032], BF, tag="lead_sb")
            g0b_sb = const.tile([128, 768], BF, tag="g0b_sb")
            gch_sb = [const.tile([128, 1280], BF, name=f"gch{g}", tag=f"gch{g}")
                      for g in range(1, 8)]
            w2_sb = const.tile([128, 1280], BF, tag="w2_sb")
            wro_sb = [const.tile([128, 4096], BF, name=f"wro{i}", tag=f"wro{i}")
                      for i in range(8)]
            # h1 split per (ko, group) for L1->L2 pipelining; h2 per ko chunk.
            h1_sb = [[const.tile([128, 512], BF, name=f"h1_{ko}_{g}",
                                 tag=f"h1_{ko}_{g}")
                      for g in range(GROUPS)] for ko in range(2)]
            h2_sb = [const.tile([128, NODES_PER], BF, name=f"h2_{fo}",
                                tag=f"h2_{fo}")
                     for fo in range(2)]

            # DMA issue order = priority order for the head of the kernel.
            nc.sync.dma_start(lead_sb[:], xw[:, 0:1032])
            nc.sync.dma_start(g0b_sb[:], xw[:, 1032:1800])
            for g in range(1, 8):
                nc.sync.dma_start(
                    gch_sb[g - 1][:],
                    xw[:, 1800 + (g - 1) * 1280:1800 + g * 1280])
            nc.sync.dma_start(w2_sb[:], w2[:])
            for i in range(8):
                nc.sync.dma_start(wro_sb[i][:], wro[:, i * 4096:(i + 1) * 4096])

            # w1 + biases ride packed inside lead/w2 (bitcast views for f32)
            w1_sb = lead_sb[:, 0:520]
            b12_sb = lead_sb[:, 512:520].bitcast(F32)
            # readout bias [bmu | blv] as a bf16 row vector (K=1 matmul lhsT)
            bias_ro_sb = w2_sb[0:1, 1024:1280]

            # PE pre-warm: dummy matmuls on memset data keep the PE busy from
            # ~1.1us so the clock ramp (HAM) completes before the first real
            # matmul arrives behind the input DMAs (~3.6us); the count is
            # tuned so the warm stream ends just as the real one begins.
            N_WARM = 22
            ones_sb = const.tile([1, 320], BF, tag="ones_sb")
            nc.gpsimd.memset(ones_sb[:], 1.0)
            warm = psum_ro.tile([128, 128], F32, tag="pro")
            for i in range(N_WARM):
                nc.tensor.matmul(warm[0:64, :], lhsT=ones_sb[:, 256:320],
                                 rhs=ones_sb[:, 0:128],
                                 start=(i == 0), stop=(i == N_WARM - 1))

            def x_cols(c0, c1):        # feature-major x slice [128, c1-c0]
                g = c0 // 512
                assert c1 <= (g + 1) * 512
                if g == 0:
                    return lead_sb[:, 520 + c0:520 + c1]
                return gch_sb[g - 1][:, c0 - g * 512:c1 - g * 512]

            def xnm_cols(b):           # node-major x block [128 n, 128 f]
                g = b // 4
                off = (b % 4) * 128
                if g == 0:
                    return g0b_sb[:, off:off + 128]
                return gch_sb[g - 1][:, 512 + off:512 + off + 128]

            def a2t_blk(b):            # [128, 128] fp8 adjacency for block b
                g = b // 4
                off = (b % 4) * 128
                if g == 0:
                    base = g0b_sb[:, 512:768].bitcast(F8)
                else:
                    base = gch_sb[g - 1][:, 1024:1280].bitcast(F8)
                return base[:, off:off + 128]

            # ---- Layer 1: aggregate-first ----
            # agg_x[f, d] = sum_s x[s, f] A[s, d] (128-dim input space),
            # then h1 = relu(W1_rel agg_x + W1_root x + b1) feature-major.
            # Software-pipelined one group ahead: the pagg -> Pool-copy ->
            # rel-matmul latency of group g hides behind group g+1's
            # root/agg matmuls.
            pf_t = [None] * GROUPS
            agg_t = [None] * GROUPS

            def l1_front(grp):
                pf_t[grp] = [psum_fm.tile([128, 512], F32,
                                          name=f"pf1_{grp}_{mo}", tag="pf")
                             for mo in range(2)]
                for mo in range(2):
                    nc.tensor.matmul(
                        pf_t[grp][mo][:],
                        lhsT=w1_sb[:, 256 + mo * 128:256 + (mo + 1) * 128],
                        rhs=x_cols(grp * 512, (grp + 1) * 512),
                        start=True, stop=False, skip_group_check=True,
                    )
                pagg = psum_hr.tile([128, 512], F32, tag="ph")
                for blk in range(4):
                    b = grp * 4 + blk
                    nc.tensor.matmul(
                        pagg[:, blk * 128:(blk + 1) * 128],
                        lhsT=xnm_cols(b), rhs=a2t_blk(b),
                        start=True, stop=True, skip_group_check=True,
                    )
                agg_t[grp] = hr_pool.tile([128, 512], BF, name=f"agg_{grp}",
                                          tag="hr")
                nc.gpsimd.tensor_copy(agg_t[grp][:], pagg[:])

            def l1_back(grp):
                for mo in range(2):
                    nc.tensor.matmul(
                        pf_t[grp][mo][:],
                        lhsT=w1_sb[:, mo * 128:(mo + 1) * 128],
                        rhs=agg_t[grp][:],
                        start=False, stop=True, skip_group_check=True,
                    )
                # relu+bias evictions split across ACT (mo=0) and DVE (mo=1)
                nc.scalar.activation(h1_sb[0][grp][:], pf_t[grp][0][:], Relu,
                                     bias=b12_sb[:, 0:1])
                nc.vector.tensor_scalar(h1_sb[1][grp][:], pf_t[grp][1][:],
                                        b12_sb[:, 1:2], 0.0, Add, Max)

            l1_front(0)
            for grp in range(GROUPS):
                if grp + 1 < GROUPS:
                    l1_front(grp + 1)
                l1_back(grp)

            # ---- Layer 2: project-first ----
            def act_cols(ko, c0, c1):
                return h1_sb[ko][c0 // 512][:, c0 % 512:c0 % 512 + (c1 - c0)]

            def emit_hr(grp):
                # two blocks share one [128,512] psum tile so one DVE copy
                # evicts both -> half the copy count
                hrs = []
                for pair in range(2):
                    ph = psum_hr.tile([128, 512], F32, tag="ph")
                    for sub in range(2):
                        b = grp * 4 + pair * 2 + sub
                        for ko in range(2):
                            nc.tensor.matmul(
                                ph[:, sub * 256:(sub + 1) * 256],
                                lhsT=act_cols(ko, b * 128, (b + 1) * 128),
                                rhs=w2_sb[:, ko * 512:ko * 512 + 256],
                                start=(ko == 0), stop=(ko == 1),
                                skip_group_check=True,
                            )
                    hr = hr_pool.tile([128, 512], BF, tag="hr")
                    nc.vector.tensor_copy(hr[:], ph[:])
                    hrs.append(hr)
                return hrs

            def emit_fm(grp, mo, hrs):
                pf = psum_fm.tile([128, 512], F32, name="pf", tag="pf")
                for ko in range(2):
                    nc.tensor.matmul(
                        pf[:],
                        lhsT=w2_sb[:, ko * 512 + 256 + mo * 128:
                                   ko * 512 + 256 + (mo + 1) * 128],
                        rhs=act_cols(ko, grp * 512, (grp + 1) * 512),
                        start=(ko == 0), stop=False,
                        skip_group_check=True,
                    )
                for blk in range(4):
                    b = grp * 4 + blk
                    nc.tensor.matmul(
                        pf[:, blk * 128:(blk + 1) * 128],
                        lhsT=hrs[blk // 2][:, (blk % 2) * 256 + mo * 128:
                                           (blk % 2) * 256 + (mo + 1) * 128],
                        rhs=a2t_blk(b),
                        start=False, stop=(blk == 3),
                        skip_group_check=True,
                    )
                nc.scalar.activation(
                    h2_sb[mo][:, grp * 512:(grp + 1) * 512], pf[:], Relu,
                    bias=b12_sb[:, 2 + mo:3 + mo],
                )

            # L2: all hr projections first, then the whole mo=0 pass before
            # mo=1 — h2_sb[0] (which gates the readout's even k-tiles)
            # completes while the PE still has the entire mo=1 pass queued.
            all_hrs = [emit_hr(grp) for grp in range(GROUPS)]
            for mo in range(2):
                for grp in range(GROUPS):
                    emit_fm(grp, mo, all_hrs[grp])

            # ---- Readout (latent-major) ----
            # pro[l, g] += sum_f wro[ft*128+f, l] * h2_fm[fo][f, g*64+n]
            # Output [128 latent, 64 graphs] per half: free dim 64 instead of
            # 256 -> half the PE cycles of the graph-major orientation.
            # Bias is folded in as a K=1 matmul against a ones vector.
            pro = psum_ro.tile([128, 128], F32, tag="pro")
            for lh in range(2):
                nc.tensor.matmul(
                    pro[:, lh * 64:(lh + 1) * 64],
                    lhsT=bias_ro_sb[:, lh * 128:(lh + 1) * 128],
                    rhs=ones_sb[0:1, 0:64],
                    start=True, stop=False, skip_group_check=True,
                )
            # ft order: (a) even ft (fo=0) of the early chunks first so the
            # readout only waits on h2_sb[0] at its head; (b) chunks 6-7 (the
            # last wro DMA arrivals) last, both halves together.
            fts = [ft for ft in range(96) if ft % 2 == 0] + \
                  [ft for ft in range(96) if ft % 2 == 1] + \
                  list(range(96, KT))
            n_mm = [0, 0]
            for ft in fts:
                n, fo = ft // 2, ft % 2
                rhs = h2_sb[fo][:, n:n + (G_PER - 1) * N_NODES + 1:N_NODES]
                for lh in range(2):
                    n_mm[lh] += 1
                    nc.tensor.matmul(
                        pro[:, lh * 64:(lh + 1) * 64],
                        lhsT=wro_sb[ft // 16][:, ((ft % 16) * 2 + lh) * 128:
                                              ((ft % 16) * 2 + lh + 1) * 128],
                        rhs=rhs,
                        start=False, stop=(n_mm[lh] == KT),
                        skip_group_check=True,
                    )
            out_sb = const.tile([128, 128], F32, tag="out_sb")
            nc.scalar.activation(out_sb[:], pro[:], Copy)
            nc.sync.dma_start(out[:], out_sb[:])

    nc.compile()
    return nc


def _get_program():
    global _PROGRAM
    if _PROGRAM is None:
        _PROGRAM = _build_program()
    return _PROGRAM


def make_in_maps(x, W1_rel, W1_root, b1, W2_rel, W2_root, b2,
                 Wmu, bmu, Wlv, blv, edge_index, batch):
    """Host-side shard + layout prep. Returns per-core input dicts."""
    x = np.asarray(x, dtype=np.float32)
    edge_index = np.asarray(edge_index)

    b12 = np.stack(
        [np.asarray(b1)[0:128], np.asarray(b1)[128:256],
         np.asarray(b2)[0:128], np.asarray(b2)[128:256]], axis=1
    ).astype(np.float32)
    w1_pack = np.concatenate(
        [np.concatenate([np.asarray(W1_rel).T, np.asarray(W1_root).T],
                        axis=1).astype(BF16),
         np.ascontiguousarray(b12).view(BF16)], axis=1)
    w2rT = np.asarray(W2_rel).T.astype(np.float32)
    w2tT = np.asarray(W2_root).T.astype(np.float32)
    bias_pack = np.zeros((128, 256), BF16)
    bias_pack[0, 0:256] = np.concatenate(
        [np.asarray(bmu), np.asarray(blv)]).astype(BF16)
    w2 = np.concatenate(
        [np.concatenate([w2rT[0:128], w2tT[0:128]], axis=1).astype(BF16),
         np.concatenate([w2rT[128:256], w2tT[128:256]], axis=1).astype(BF16),
         bias_pack], axis=1)
    # [128 f, KT*2*128]: per-(ft, lh) stationary [128, 128] tiles, ft-major
    wro_cat = np.concatenate([np.asarray(Wmu).T, np.asarray(Wlv).T], axis=1)
    wro = np.ascontiguousarray(
        wro_cat.reshape(KT, 128, 2, 128).transpose(1, 0, 2, 3)
        .reshape(128, KT * 256)
    ).astype(BF16)

    # Dense per-2-graph-block adjacency counts: A[blk][s, d] = #edges s->d.
    src = edge_index[0].astype(np.int64)
    dst = edge_index[1].astype(np.int64)
    blk = dst >> 7                       # 128 nodes per 2-graph block
    s_loc = src - (blk << 7)
    d_loc = dst - (blk << 7)
    # edges are intra-graph by construction; fail loudly rather than let a
    # cross-block index wrap around in np.add.at
    assert s_loc.min() >= 0 and s_loc.max() < 128, "edge crosses graph block"
    A = np.zeros((BS // 2, 128, 128), np.float32)
    np.add.at(A, (blk, s_loc, d_loc), 1.0)
    assert A.max() <= 16, "edge count not exactly representable in fp8e4"

    in_maps = []
    for c in range(N_CORES):
        xs = x[c * NODES_PER:(c + 1) * NODES_PER]
        xT = np.ascontiguousarray(xs.T).astype(BF16)               # [128, 4096]
        xnm = np.ascontiguousarray(
            xs.reshape(BLOCKS, 128, IN_F).transpose(1, 0, 2)
            .reshape(128, BLOCKS * IN_F)).astype(BF16)             # [128, 4096]
        Ac = A[c * BLOCKS:(c + 1) * BLOCKS]
        a2t8 = np.ascontiguousarray(
            Ac.transpose(1, 0, 2).reshape(128, BLOCKS * 128)
        ).astype(F8E4).view(BF16)                                  # [128, 2048]
        parts = [w1_pack, xT[:, 0:512], xnm[:, 0:512], a2t8[:, 0:256]]
        for g in range(1, 8):
            parts += [xT[:, g * 512:(g + 1) * 512],
                      xnm[:, g * 512:(g + 1) * 512],
                      a2t8[:, g * 256:(g + 1) * 256]]
        xw = np.ascontiguousarray(np.concatenate(parts, axis=1))
        in_maps.append(dict(xw=xw, w2=w2, wro=wro))
    return in_maps


def kernel(**inputs):
    from concourse.bass_utils import run_bass_kernel_spmd

    nc = _get_program()
    in_maps = make_in_maps(**inputs)
    res = run_bass_kernel_spmd(nc, in_maps, list(range(N_CORES)))
    # per-core out is [128 latent, 64 mu-graphs | 64 lv-graphs]
    mu = np.concatenate(
        [res.results[c]["out"][:, 0:G_PER].T for c in range(N_CORES)], axis=0)
    logvar = np.concatenate(
        [res.results[c]["out"][:, G_PER:128].T for c in range(N_CORES)], axis=0)
    return np.ascontiguousarray(mu, np.float32), \
        np.ascontiguousarray(logvar, np.float32)
